# revision 4
# baseline (speedup 1.0000x reference)
"""Differential attention kernel for Trainium2 (8 NeuronCores).

Algebraic rewrite: out = (attn1 - lam*attn2) @ (x @ Wv) @ Wo
                       = diff_attn @ (x @ W_eff),  W_eff = Wv @ Wo [512,512].
W_eff depends only on weights, so it is folded on the HOST (same as the
bv/bo bias fold) -- no on-device 8192-wide contraction and, critically,
no AllReduce (any collective costs >=28us of fixed overhead).

Second reassociation: out = (p @ x) @ W_eff, so the big contraction
(p @ x over 2048 keys) feeds a tiny [512,512] projection.

Sharding: 8 cores = 2 batches x 4 q-chunks (512 queries each), fully
data-parallel, zero collectives. Each core: K-side qkv for its batch
(duplicated across the 4 cores of a batch -- cheaper than any exchange),
Q-side qkv for its chunk, then a per-128-query-block pipeline:
scores -> exp/softmax sums (ACT) -> combine (DVE) -> PE transpose ->
pxT = (p @ x)^T -> out_qb = px @ W_eff.

Scheduling: the K-side projection chains are interleaved with qb0's
score chains so ACT exp work starts early; in the steady loop the
previous block's post work (transposes, pxT, final) is split into PE
quanta slotted between individual score chains, so PE has ready work
while ACT drains the exp evictions. PSUM evictions that gate nothing
run on DVE to keep ACT's in-order queue exp-only.

Numerics: scores Q/K in fp32r (full-speed on PE for free dim >=256);
x, e^s, p, W_eff, out bf16. 1/sum(e1) folds into the final PSUM
eviction; bv/bo fold into a host-side constant bias using
sum_k(diff_attn[q,:]) == 1 - lam.
"""

import math
from contextlib import ExitStack

import numpy as np
import ml_dtypes

import concourse.bass as bass
from concourse import bacc
import concourse.mybir as mybir
import concourse.tile as tile
from concourse import bass_utils
from concourse.bass import ts, ds
from concourse.masks import make_identity

# Problem shapes (hardcoded per harness contract).
B = 2
S = 2048
D = 512
DM = 512             # output dim
P = 128
QC = 512             # q-chunk per core
SCALE = 1.0 / math.sqrt(64.0)
LAMBDA_INIT = 0.8
LAYER_INDEX = 0

F32 = mybir.dt.float32
F32R = mybir.dt.float32r
BF16 = mybir.dt.bfloat16
EXP = mybir.ActivationFunctionType.Exp
IDENT = mybir.ActivationFunctionType.Identity
AXX = mybir.AxisListType.X
MUL = mybir.AluOpType.mult
ADD = mybir.AluOpType.add

KD = D // P          # 4 contraction chunks of the model dim
MQ = (2 * D) // P    # 8 m-chunks of the qkv output dim (Q1 Q1 Q2 Q2 K1 K1 K2 K2)
SN = S // 512        # 4 free chunks of S
QB = QC // P         # 4 q-blocks per chunk
NKC = S // P         # 16 k-chunks of 128


class Pools:
    pass


def open_pools(tc, ctx):
    po = Pools()
    po.persist = ctx.enter_context(tc.tile_pool(name="persist", bufs=1))
    po.xp = ctx.enter_context(tc.tile_pool(name="xp", bufs=1))
    po.xsp = ctx.enter_context(tc.tile_pool(name="xsp", bufs=1))
    po.wefp = ctx.enter_context(tc.tile_pool(name="wefp", bufs=1))
    po.e1p = ctx.enter_context(tc.tile_pool(name="e1p", bufs=3))
    po.e2p = ctx.enter_context(tc.tile_pool(name="e2p", bufs=2))
    po.smallp = ctx.enter_context(tc.tile_pool(name="smallp", bufs=3))
    po.r1p = ctx.enter_context(tc.tile_pool(name="r1p", bufs=2 * QB))
    po.pbp = ctx.enter_context(tc.tile_pool(name="pbp", bufs=2))
    po.ptp = ctx.enter_context(tc.tile_pool(name="ptp", bufs=2))
    po.pxp = ctx.enter_context(tc.tile_pool(name="pxp", bufs=2))
    po.ofp = ctx.enter_context(tc.tile_pool(name="ofp", bufs=2))
    # PSUM: 5 (scores/qkv) + 2 (pxT) + 1 (final) = 8 banks
    po.qps = ctx.enter_context(tc.tile_pool(name="qps", bufs=5, space="PSUM"))
    po.pps = ctx.enter_context(tc.tile_pool(name="pps", bufs=2, space="PSUM"))
    po.fps = ctx.enter_context(tc.tile_pool(name="fps", bufs=1, space="PSUM"))
    return po


def emit_const_setup(tc, po):
    nc = tc.nc
    ident_f32 = po.persist.tile([P, P], F32, name="ident_f32")
    po.ident = po.persist.tile([P, P], BF16, name="ident")
    make_identity(nc, ident_f32)
    nc.vector.tensor_copy(po.ident, ident_f32)
    # Touch Exp now so ACT's LoadActFuncSet (~1.3us) runs during the DMA
    # head instead of stalling the first real exp eviction.
    warm = po.persist.tile([P, 1], F32, name="act_warm")
    nc.scalar.activation(warm, ident_f32[:, 0:1], EXP)


def emit_iter(tc, po, i, xT, xr, wq, wef_d, lam, bq, out):
    nc = tc.nc

    # ---- DMA loads (first-consumption order, all before any store) ----
    wq_sb = po.persist.tile([P, KD, 2 * D], BF16, tag="wq", name=f"wq_{i}")
    wqr = wq.rearrange("(c p) m -> p c m", p=P)
    xbf = po.xp.tile([P, KD, S], BF16, tag="xbf", name=f"xbf_{i}")
    xTr = xT.rearrange("(c p) s -> p c s", p=P)
    nc.sync.dma_start(wq_sb[:, :, 0:P], wqr[:, :, 0:P])          # m0
    nc.sync.dma_start(xbf[:, 0:2, ts(0, 512)], xTr[:, 0:2, ts(0, 512)])
    nc.sync.dma_start(xbf[:, 2:4, ts(0, 512)], xTr[:, 2:4, ts(0, 512)])
    nc.sync.dma_start(wq_sb[:, :, P:D], wqr[:, :, P:D])          # m1-3
    bq_sb = po.smallp.tile([P, MQ], F32, tag="bq", name=f"bq_{i}")
    lam_sb = po.smallp.tile([P, 1], F32, tag="lam", name=f"lam_{i}")
    nc.sync.dma_start(bq_sb, bq)
    nc.sync.dma_start(wq_sb[:, :, D:], wqr[:, :, D:])            # K half
    nc.sync.dma_start(xbf[:, :, ts(1, 512)], xTr[:, :, ts(1, 512)])
    nc.sync.dma_start(lam_sb, lam)
    for sn in range(2, SN):
        nc.sync.dma_start(xbf[:, :, ts(sn, 512)], xTr[:, :, ts(sn, 512)])
    xsb = po.xsp.tile([P, NKC, D], BF16, tag="xsb", name=f"xsb_{i}")
    nc.sync.dma_start(xsb, xr.rearrange("(c p) m -> p c m", p=P))
    wef = po.wefp.tile([P, KD, DM], BF16, tag="wef", name=f"wef_{i}")
    nc.sync.dma_start(wef, wef_d.rearrange("(c p) m -> p c m", p=P))

    # ---- Q-side qkv: this chunk's queries are cols 0:QC of rolled x ----
    qkvT_Q = po.persist.tile([P, 4, QC], F32R, tag="qkvQ", name=f"qkvQ_{i}")
    for m in range(4):
        pt = po.qps.tile([P, 512], F32, tag="ps", name=f"qq_{i}_{m}")
        for dc in range(KD):
            nc.tensor.matmul(
                pt, wq_sb[:, dc, ts(m, P)], xbf[:, dc, 0:QC],
                start=(dc == 0), stop=(dc == KD - 1))
        nc.vector.tensor_scalar_add(qkvT_Q[:, m], pt, bq_sb[:, m : m + 1])

    qkvT_K = po.persist.tile([P, 4, S], F32R, tag="qkvK", name=f"qkvK_{i}")
    st = Pools()
    st.r1s = []
    sums_by_qb = {}
    ets_by_qb = {}

    def emit_kside(sn):
        for m in range(4, MQ):
            pt = po.qps.tile([P, 512], F32, tag="ps", name=f"qk_{i}_{sn}_{m}")
            for dc in range(KD):
                nc.tensor.matmul(
                    pt, wq_sb[:, dc, ts(m, P)], xbf[:, dc, ts(sn, 512)],
                    start=(dc == 0), stop=(dc == KD - 1))
            nc.vector.tensor_scalar_add(qkvT_K[:, m - 4, ts(sn, 512)], pt,
                                        bq_sb[:, m : m + 1])

    def emit_chain(qb, mi, kn):
        """One score chain + exp eviction for (q-block, map, key-slice)."""
        if qb not in ets_by_qb:
            e1 = po.e1p.tile([P, S], BF16, tag="e0", name=f"e0_{i}_{qb}")
            e2 = po.e2p.tile([P, S], BF16, tag="e1", name=f"e1_{i}_{qb}")
            s1 = po.smallp.tile([P, SN], F32, tag="sum0", name=f"sm0_{i}_{qb}")
            s2 = po.smallp.tile([P, SN], F32, tag="sum1", name=f"sm1_{i}_{qb}")
            ets_by_qb[qb] = (e1, e2)
            sums_by_qb[qb] = (s1, s2)
        et = ets_by_qb[qb][mi]
        stt = sums_by_qb[qb][mi]
        pt = po.qps.tile([P, 512], F32, tag="ps", name=f"ps_{i}_{qb}_{mi}_{kn}")
        for dc in range(2):
            nc.tensor.matmul(
                pt,
                qkvT_Q[:, 2 * mi + dc, ts(qb, P)],
                qkvT_K[:, 2 * mi + dc, ts(kn, 512)],
                start=(dc == 0), stop=(dc == 1))
        nc.scalar.activation(
            et[:, ts(kn, 512)], pt, EXP, scale=SCALE,
            accum_out=stt[:, kn : kn + 1])

    def emit_norms(qb):
        """Normalizer chain (DVE) once both maps' sums are in flight."""
        sums = sums_by_qb[qb]
        s1 = po.smallp.tile([P, 1], F32, tag="s1", name=f"s1_{i}_{qb}")
        nc.vector.reduce_sum(s1, sums[0], axis=AXX)
        r1 = po.r1p.tile([P, 1], F32, tag="r1", name=f"r1_{i}_{qb}")
        nc.vector.reciprocal_approx_fast(r1, s1)
        st.r1s.append(r1)
        s2 = po.smallp.tile([P, 1], F32, tag="s2", name=f"s2_{i}_{qb}")
        nc.vector.reduce_sum(s2, sums[1], axis=AXX)
        r2 = po.smallp.tile([P, 1], F32, tag="r2", name=f"r2_{i}_{qb}")
        nc.vector.reciprocal_approx_fast(r2, s2)
        # lam_sb holds -lam, so r2q = -lam*s1/s2 and the combine is a
        # single fused multiply-add: p = e2*r2q + e1.
        u = po.smallp.tile([P, 1], F32, tag="u", name=f"u_{i}_{qb}")
        nc.vector.tensor_mul(u, s1, lam_sb)
        r2q = po.smallp.tile([P, 1], F32, tag="r2q", name=f"r2q_{i}_{qb}")
        nc.vector.tensor_mul(r2q, u, r2)
        return r2q

    def emit_combine(qb, r2q, last=False):
        """p = e1 + r2q*e2, kn slices alternating DVE / GpSimd, then p^T
        into ptile (key k -> chunk k//128, partition k%128). Mid-kernel
        blocks use the xbar DMA-transpose (off the PE); the last block uses
        PE transposes to dodge the 900ns DMA-sem latency on the tail."""
        ets = ets_by_qb[qb]
        pb = po.pbp.tile([P, S], BF16, tag="pb", name=f"pb_{i}_{qb}")
        tm = po.pbp.tile([P, 1024], BF16, tag="tm", name=f"tm_{i}_{qb}")
        ptile = po.ptp.tile([P, NKC, P], BF16, tag="pt", name=f"pt_{i}_{qb}")
        # two-op combine stays on DVE's 2-byte fast paths ((mult,add) fused
        # form runs at 1x): tmp = e2*r2q at 4x, pb = tmp + e1 at 2x
        for h in range(2):
            hs = ts(h, 1024)
            nc.vector.tensor_scalar_mul(tm, ets[1][:, hs], r2q)
            nc.vector.tensor_add(pb[:, hs], tm, ets[0][:, hs])
            nc.sync.dma_start_transpose(
                ptile[:, ts(h, 8), :], pb[:, hs])
        return ptile

    def post_quanta(qb, ptile, last=False):
        """PE work for one finished q-block, split into fill quanta."""
        px = po.pps.tile([P, KD, P], F32, tag="px", name=f"px_{i}_{qb}")

        def pxq(db):
            def go():
                for kc in range(NKC):
                    nc.tensor.matmul(
                        px[:, db], xsb[:, kc, ts(db, P)], ptile[:, kc, :],
                        start=(kc == 0), stop=(kc == NKC - 1))
            return go

        def fin():
            pxT = po.pxp.tile([P, KD, P], BF16, tag="pxT",
                              name=f"pxT_{i}_{qb}")
            # split the eviction across DVE and ACT halves
            nc.vector.tensor_copy(pxT[:, 0:2], px[:, 0:2])
            nc.scalar.activation(pxT[:, 2:4], px[:, 2:4], IDENT)
            ofsb = po.ofp.tile([P, DM], BF16, tag="of", name=f"of_{i}_{qb}")
            if not last:
                ft = po.fps.tile([P, DM], F32, tag="f", name=f"ft_{i}_{qb}")
                for dc in range(KD):
                    nc.tensor.matmul(
                        ft, pxT[:, dc], wef[:, dc, :],
                        start=(dc == 0), stop=(dc == KD - 1))
                nc.vector.tensor_scalar_mul(ofsb, ft, st.r1s[qb])
                nc.sync.dma_start(out[ds(qb * P, P), :], ofsb)
            else:
                # last block: column-halved final so eviction/DMA pipeline
                ft = po.fps.tile([P, 2, 256], F32, tag="f",
                                 name=f"ft_{i}_{qb}")
                for h in range(2):
                    hs = ts(h, 256)
                    for dc in range(KD):
                        nc.tensor.matmul(
                            ft[:, h], pxT[:, dc], wef[:, dc, hs],
                            start=(dc == 0), stop=(dc == KD - 1))
                    eng = nc.vector if h == 0 else nc.scalar
                    if h == 0:
                        nc.vector.tensor_scalar_mul(ofsb[:, hs], ft[:, h],
                                                    st.r1s[qb])
                    else:
                        nc.scalar.activation(ofsb[:, hs], ft[:, h], IDENT,
                                             scale=st.r1s[qb])
                    nc.sync.dma_start(out[ds(qb * P, P), hs], ofsb[:, hs])

        return [pxq(0), pxq(1), pxq(2), pxq(3), fin]

    # ---- front: K-side chains interleaved with qb0/qb1/qb2-mi0 chains ----
    for sn in range(SN):
        emit_kside(sn)
        for qb in (0, 1):
            emit_chain(qb, 0, sn)
            emit_chain(qb, 1, sn)
        emit_chain(2, 0, sn)
    r2qs = {0: emit_norms(0), 1: emit_norms(1)}

    # ---- steady: remaining chains interleaved with posts of qb0/qb1 ----
    for qb in (2, 3):
        pqb = qb - 2
        ptile = emit_combine(pqb, r2qs[pqb])
        quanta = post_quanta(pqb, ptile)
        slots = [(1, kn) for kn in range(SN)] if qb == 2 else [
            (mi, kn) for mi in range(2) for kn in range(SN)]
        for si, (mi, kn) in enumerate(slots):
            emit_chain(qb, mi, kn)
            if (qb == 2 or si not in (0, 2, 4)) and quanta:
                quanta.pop(0)()
        r2qs[qb] = emit_norms(qb)
        for q in quanta:
            q()

    # ---- drain: posts of qb2/qb3, px chains interleaved so the pxT
    # eviction latency of each block hides under the other's matmuls ----
    pt2 = emit_combine(2, r2qs[2])
    pt3 = emit_combine(3, r2qs[3])
    q2 = post_quanta(2, pt2)
    q3 = post_quanta(3, pt3, last=True)
    for q in q2[:4]:
        q()
    q3[0]()
    q2[4]()          # fin(2) while px(3) runs
    for q in q3[1:]:
        q()


def build_module(n_iters=1, phases="full"):
    nc = bacc.Bacc("TRN2", target_bir_lowering=False, debug=False)
    xT = nc.dram_tensor("xT", (D, S), BF16, kind="ExternalInput").ap()
    xr = nc.dram_tensor("xr", (S, D), BF16, kind="ExternalInput").ap()
    wq = nc.dram_tensor("wq", (D, 2 * D), BF16, kind="ExternalInput").ap()
    wef_d = nc.dram_tensor("wef", (D, DM), BF16, kind="ExternalInput").ap()
    lam = nc.dram_tensor("lam", (P, 1), F32, kind="ExternalInput").ap()
    bq = nc.dram_tensor("bq", (P, MQ), F32, kind="ExternalInput").ap()
    out = nc.dram_tensor("out", (QC, DM), BF16, kind="ExternalOutput").ap()
    with tile.TileContext(nc) as tc, ExitStack() as ctx:
        po = open_pools(tc, ctx)
        emit_const_setup(tc, po)
        for i in range(n_iters):
            emit_iter(tc, po, i, xT, xr, wq, wef_d, lam, bq, out)
    nc.compile()
    return nc


_NC = None


def _get_module():
    global _NC
    if _NC is None:
        _NC = build_module()
    return _NC


def host_prep(**inputs):
    """Host-side input prep: returns (in_maps, lam, host_bias)."""
    x = np.asarray(inputs["x"], np.float32)
    Wqkv = np.asarray(inputs["Wqkv"], np.float32)
    bqkv = np.asarray(inputs["bqkv"], np.float32)
    Wv = np.asarray(inputs["Wv"], np.float32)
    bv = np.asarray(inputs["bv"], np.float32)
    Wo = np.asarray(inputs["Wo"], np.float32)
    bo = np.asarray(inputs["bo"], np.float32)
    lq1 = np.asarray(inputs["lq1"], np.float32)
    lk1 = np.asarray(inputs["lk1"], np.float32)
    lq2 = np.asarray(inputs["lq2"], np.float32)
    lk2 = np.asarray(inputs["lk2"], np.float32)

    lam = float(
        np.exp(np.sum(lq1 * lk1, dtype=np.float32))
        - np.exp(np.sum(lq2 * lk2, dtype=np.float32))
        + (LAMBDA_INIT - 0.6 * math.exp(-0.3 * LAYER_INDEX))
    )
    bq_host = np.ascontiguousarray(bqkv.reshape(MQ, P).T)
    # device gets -lam so the combine is a fused multiply-add
    lam_host = np.full((P, 1), -lam, np.float32)
    bf = ml_dtypes.bfloat16
    wq_host = Wqkv.astype(bf)
    # weight-only fold: W_eff = Wv @ Wo, computed once on host in fp32
    wef_host = (Wv @ Wo).astype(bf)

    in_maps = []
    for c in range(8):
        b, qc = divmod(c, 4)
        # Roll x's sequence dim so this core's q-chunk occupies the first QC
        # columns of xT. K-side scores and the p@x contraction use the same
        # rolled order, so the roll cancels.
        xroll = np.roll(x[b], -qc * QC, axis=0)
        in_maps.append({
            "xT": np.ascontiguousarray(xroll.T).astype(bf),
            "xr": xroll.astype(bf),
            "wq": wq_host,
            "wef": wef_host,
            "lam": lam_host,
            "bq": bq_host,
        })
    # sum_k diff_attn[q, :] == 1 - lam exactly, so bv and bo fold into a
    # constant per-output-column correction.
    host_bias = ((1.0 - lam) * bv) @ Wo + bo
    return in_maps, lam, host_bias.astype(np.float32)


def kernel(**inputs):
    in_maps, _lam, host_bias = host_prep(**inputs)
    nc = _get_module()
    res = bass_utils.run_bass_kernel_spmd(nc, in_maps, core_ids=list(range(8)))
    out = np.empty((B, S, DM), np.float32)
    for c in range(8):
        b, qc = divmod(c, 4)
        out[b, qc * QC : (qc + 1) * QC, :] = (
            res.results[c]["out"].astype(np.float32))
    out += host_bias
    return out
